# revision 46
# baseline (speedup 1.0000x reference)
"""DynamicDownsampling Trainium2 kernel.

out[b,c,h,w] = sum_{t,ki,kj} softmax(k)[b,h,w,t,ki,kj] * xpad[b,c,t,2h+ki,2w+kj]
             / (same with DT + 1e-8)

Softmax normalization cancels between numerator and denominator:
  out = sum(e * x_win) / sum(e * (DT_win + 1e-8)),  e = exp(kernel)

Sharding: 8 cores = 2 batches x 4 h-quarters (80 output rows each).

Layout (v4, block-grid + fp16 products): each SBUF partition owns one
5x40-output spatial tile (16x8 grid = 128 partitions); its input patch
(12 rows x 4 kj-phase column planes x 40) lives in the free dim in fp16,
so every tap (t,ki,kj) is a stride-1, 4-byte-aligned view -> the DVE
multiplies run in 2x packed mode. exp(kernel) runs on the ACT engine
(fp32 in, fp16 out). The PE accumulates fp16 products into fp32 PSUM via
identity matmul at full rate. Final divide in fp32.

Error budget: fp16 product rounding ~5e-4/term, fp32 accumulate; end-to-end
scale-relative absmax ~2e-4.
"""

import sys

for _p in ("/root/.axon_site", "/root/.axon_site/_ro/trn_rl_repo",
           "/root/.axon_site/_ro/pypackages", "/opt/trn_rl_repo"):
    if _p not in sys.path:
        sys.path.append(_p)

import numpy as np

import concourse.bacc as bacc
import concourse.bass as bass
import concourse.mybir as mybir
import concourse.tile as tile
from concourse import bass_utils

# Problem constants (hardcoded).
B, C, T, H, W = 2, 3, 3, 640, 640
K, S = 4, 2
HO, WO = H // 2, W // 2             # 320
N_CORES = 8
HQ = HO // 4                        # 80 output rows per core
ROWS_IN = 2 * HQ + 2                # 162 padded input rows per core
WIN = W + 2                         # 642 padded cols
F32 = mybir.dt.float32
F16 = mybir.dt.float16

NBH, NBW = 16, 8                    # partition grid: p = bh*8 + bw
TH, TW = HQ // NBH, WO // NBW       # 5 x 40 outputs per partition
RI = 2 * TH + 2                     # 12 patch rows per partition
NIMG = 4                            # 3 x channels + DT
TAPS = K * K * T                    # 48
FD = NIMG * TH * TW                 # 800 product elements per partition/tap
ECH = 8 # kernel exp chunk (taps)

# taps whose multiply runs on GPSIMD (Pool) instead of DVE; mid-range taps
# balance the engines while DVE owns the head (DMA-gated) and the tail
GPSIMD_TAPS = frozenset({6, 13, 20, 27, 34, 41})


def _build_program():
    nc = bacc.Bacc("TRN2", target_bir_lowering=False, debug=False,
                   num_devices=N_CORES)

    # Host-pre-arranged (see _shard_inputs):
    #  xh[t, p, img, ri, kj, tw] fp16 (kj-phase column planes)
    #  eg[p, tap, th, tw] fp32, ident[128,128] fp32
    xh = nc.dram_tensor("xh", [T, 128, NIMG, RI, K, TW], F16,
                        kind="ExternalInput").ap()
    eg = nc.dram_tensor("eg", [128, TAPS, TH, TW], F32,
                        kind="ExternalInput").ap()
    ident = nc.dram_tensor("ident", [128, 128], F32, kind="ExternalInput").ap()
    out = nc.dram_tensor("out", [128, C, TH, TW], F32,
                         kind="ExternalOutput").ap()

    with tile.TileContext(nc) as tc:
        with (
            tc.tile_pool(name="xp", bufs=1) as xpool,
            tc.tile_pool(name="ep", bufs=1) as epool,
            tc.tile_pool(name="pp", bufs=8) as ppool,
            tc.tile_pool(name="esp", bufs=2) as espool,
            tc.tile_pool(name="sp", bufs=1) as spool,
            tc.tile_pool(name="psum", bufs=1, space="PSUM") as psum,
        ):
            idt0 = spool.tile([128, 128], F32)
            idt = spool.tile([128, 128], F16, tag="idth")

            et = epool.tile([128, TAPS, TH * TW], F16)
            xts = []
            def load_e_chunk(c0, n=ECH):
                es = espool.tile([128, ECH, TH * TW], F32, tag="es")
                nc.sync.dma_start(
                    out=es[:, 0:n, :],
                    in_=eg[:, c0:c0 + n, :, :].rearrange(
                        "p t a b -> p t (a b)"))
                # exp: fp32 kernel in, fp16 weights out
                nc.scalar.activation(
                    out=et[:, c0:c0 + n, :], in_=es[:, 0:n, :],
                    func=mybir.ActivationFunctionType.Exp)

            def load_x(t):
                xt = xpool.tile([128, NIMG, RI, K, TW], F16, tag=f"x{t}")
                nc.sync.dma_start(out=xt[:, :, :, :, :], in_=xh[t])
                xts.append(xt)

            for t in range(T):
                # t0: small first chunk, then x (shortest path to the first
                # multiply), then the rest; t1/t2: chunks ahead of the x
                c0 = t * K * K
                if t == 0:
                    load_e_chunk(c0)
                    load_x(t)
                    nc.sync.dma_start(out=idt0[:, :], in_=ident[:, :])
                    nc.vector.tensor_copy(idt[:, :], idt0[:, :])
                    load_e_chunk(c0 + ECH)
                else:
                    load_e_chunk(c0)
                    load_e_chunk(c0 + ECH)
                    load_x(t)

            acc = psum.tile([128, FD], F32)
            for t in range(T):
                for ki in range(K):
                    for kj in range(K):
                        tap = t * K * K + ki * K + kj
                        base = xts[t][:, :, :, :, :]
                        in0 = bass.AP(
                            tensor=base.tensor,
                            offset=base.offset + ki * K * TW + kj * TW,
                            ap=[list(base.ap[0]), [RI * K * TW, NIMG],
                                [2 * K * TW, TH], [1, TW]])
                        ebase = et[:, tap, :]
                        in1 = bass.AP(
                            tensor=ebase.tensor, offset=ebase.offset,
                            ap=[list(ebase.ap[0]), [0, NIMG], [1, TH * TW]])
                        prod = ppool.tile([128, NIMG, TH * TW], F16, tag="pr")
                        if tap in GPSIMD_TAPS:
                            nc.gpsimd.tensor_tensor(
                                prod[:, :, :], in0, in1,
                                op=mybir.AluOpType.mult)
                        else:
                            nc.vector.tensor_mul(prod[:, :, :], in0, in1)

                        rhs = prod[:, :, :].rearrange("p a b -> p (a b)")
                        for f0 in range(0, FD, 512):
                            f1 = min(f0 + 512, FD)
                            nc.tensor.matmul(
                                acc[:, f0:f1], idt[:, :], rhs[:, f0:f1],
                                start=(tap == 0), stop=(tap == TAPS - 1))

            # ---- divide + store (fp32) ----
            sden = C * TH * TW
            r = spool.tile([128, TH * TW], F32)
            nc.vector.reciprocal(out=r[:, :], in_=acc[:, sden:FD])
            o = spool.tile([128, C, TH * TW], F32)
            rv = r[:, :]
            rb = bass.AP(tensor=rv.tensor, offset=rv.offset,
                         ap=[list(rv.ap[0]), [0, C], [1, TH * TW]])
            nc.vector.tensor_mul(
                o[:, :, :], acc[:, 0:sden].rearrange("p (c f) -> p c f", c=C),
                rb)
            nc.sync.dma_start(
                out=out[:, :, :, :].rearrange("p c a b -> p c (a b)"),
                in_=o[:, :, :])

    nc.compile()
    return nc


_NC_CACHE = None


def _shard_inputs(x: np.ndarray, kernel: np.ndarray, DT: np.ndarray):
    x = np.asarray(x, dtype=np.float32)
    kern = np.asarray(kernel, dtype=np.float32)
    dt = np.asarray(DT, dtype=np.float32)

    pad = (K - S) // 2
    xp = np.pad(x, ((0, 0),) * 3 + ((pad, pad), (pad, pad)), mode="edge")
    dtp = np.pad(dt, ((0, 0),) * 3 + ((pad, pad), (pad, pad)), mode="edge")
    dtp = (dtp + 1e-8).astype(np.float32)

    ident = np.eye(128, dtype=np.float32)
    swv = np.lib.stride_tricks.sliding_window_view
    in_maps = []
    for core in range(N_CORES):
        b, q = divmod(core, 4)
        r0 = 2 * HQ * q
        arr = np.concatenate(
            [xp[b, :, :, r0:r0 + ROWS_IN, :], dtp[b, :, :, r0:r0 + ROWS_IN, :]],
            axis=0).astype(np.float16)          # [img, t, rows, cols]
        # xh[t, p=(bh,bw), img, ri, kj, w']: col = 2*(40*bw+w') + kj
        xhc = np.empty((T, NBH, NBW, NIMG, RI, K, TW), np.float16)
        for kj in range(K):
            colsel = arr[:, :, :, kj:kj + 2 * WO:2]          # [img,t,162,320]
            rw = swv(colsel, RI, axis=2)[:, :, ::2 * TH]     # [img,t,16,320,12]
            rw = rw.reshape(NIMG, T, NBH, NBW, TW, RI)
            xhc[:, :, :, :, :, kj, :] = rw.transpose(1, 2, 3, 0, 5, 4)
        xhr = np.ascontiguousarray(
            xhc.reshape(T, 128, NIMG, RI, K, TW))
        # eg[p, tap=(t,kk), th, tw] fp32
        ksl = kern[b, :, :, HQ * q:HQ * (q + 1), :]          # [16,3,80,320]
        egr = (ksl.reshape(K * K, T, NBH, TH, NBW, TW)
               .transpose(2, 4, 1, 0, 3, 5)
               .reshape(128, TAPS, TH, TW))
        in_maps.append({"xh": xhr, "eg": np.ascontiguousarray(egr),
                        "ident": ident})
    return in_maps


def kernel(x: np.ndarray, kernel: np.ndarray, DT: np.ndarray) -> np.ndarray:
    global _NC_CACHE
    in_maps = _shard_inputs(x, kernel, DT)
    if _NC_CACHE is None:
        _NC_CACHE = _build_program()
    res = bass_utils.run_bass_kernel_spmd(
        _NC_CACHE, in_maps, core_ids=list(range(N_CORES)))

    out = np.empty((B, C, HO, WO), dtype=np.float32)
    for core in range(N_CORES):
        b, q = divmod(core, 4)
        o = (res.results[core]["out"]
             .reshape(NBH, NBW, C, TH, TW)
             .transpose(2, 0, 3, 1, 4)
             .reshape(C, HQ, WO))
        out[b, :, HQ * q:HQ * (q + 1), :] = o
    return out
